# revision 24
# baseline (speedup 1.0000x reference)
"""Trainium2 Bass kernel for multi-head causal attention.

Problem: B=2, S=2048, D=1024, H=16 heads (head_dim=64), fp32.
  q,k,v = x@Wq, x@Wk, x@Wv  (per-head split)
  scores = q@k^T, causal mask, softmax(scores/sqrt(64))
  out = (attn@v concat) @ Wo + bo

Sharding (8 cores): core c -> batch b=c//4, head group g=c%4 (4 heads).
Each core computes its 4 heads' attention plus the partial output
projection (row-parallel Wo); host sums 4 partials per batch and adds bo.

Layout strategy (zero on-device transposes):
 - x^T passed host-transposed (feature-major).
 - Q^T,K^T produced feature-major: (head_dim x tokens), two heads stacked
   per 128-partition tile; scores^T computed per 64-partition row group.
 - Both heads' scores^T tiles (k x q) land in one 2-bank PSUM tile so the
   causal mask memset + exp run as single wide instructions. The exp'd
   bf16 tile is directly the PV stationary operand. V is token-major with
   an appended ones-column so the PV matmul also emits the softmax
   denominators.
 - stage-and-release normalization: accumulators are copied to SBUF the
   moment a pair finishes (freeing PSUM banks in ~1us); the fp32 chain
   (partition-0 bounce -> fast reciprocal -> gpsimd partition broadcast
   -> DVE multiply) then runs off the critical path.
 - software-pipelined emission: projections for range r+1 are emitted
   before attention(r) and the output projection runs one range behind,
   so the PE never stalls on the softmax chain and the PE activity
   monitor stays at full clock.
Matmul operands are bf16 (1 cycle/row PE rate); every accumulation and
the softmax normalization stay fp32 in PSUM.
"""

import sys

sys.path.insert(0, "/opt/trn_rl_repo")

import ml_dtypes
import numpy as np

import concourse.bass as bass  # noqa: F401
import concourse.tile as tile
from concourse import bacc, bass_utils, mybir

F32 = mybir.dt.float32
MMDT = mybir.dt.bfloat16
NPDT = ml_dtypes.bfloat16
EXPF = mybir.ActivationFunctionType.Exp

B, S, D, H, HD = 2, 2048, 1024, 16, 64
N_CORES = 8
HPC = 4            # heads per core
GW = HPC * HD      # head-group width per core = 256
SCALE = 1.0 / np.sqrt(HD)
NEG = -1.0e30

_CACHE = {}
LAST_RESULTS = None


def _maybe_install_trace_hook():
    """If BASS_TRACE is set, bass_utils needs antenv.axon_hooks (absent in
    this image). Install it from trn_boot when possible; otherwise disable
    tracing so the run still works."""
    import os

    if not os.environ.get("BASS_TRACE"):
        return
    try:
        import antenv.axon_hooks  # noqa: F401
        return
    except ImportError:
        pass
    try:
        import types

        from trn_agent_boot.trn_boot import _ntff_profile_via_ctypes

        hook = _ntff_profile_via_ctypes("/opt/axon/libaxon_pjrt.so")
        mod = types.ModuleType("antenv.axon_hooks")
        mod.get_axon_ntff_profile_hook = lambda: hook
        mod.set_axon_ntff_profile_hook = lambda h: None
        import antenv

        sys.modules["antenv.axon_hooks"] = mod
        antenv.axon_hooks = mod
    except Exception:
        os.environ["BASS_NEVER_TRACE"] = "1"


def _build():
    nc = bacc.Bacc("TRN2", target_bir_lowering=False, debug=False)

    xT = nc.dram_tensor("xT", [D, S], MMDT, kind="ExternalInput").ap()
    wq = nc.dram_tensor("wq", [128, D // 128 * GW], MMDT, kind="ExternalInput").ap()
    wk = nc.dram_tensor("wk", [128, D // 128 * GW], MMDT, kind="ExternalInput").ap()
    wv = nc.dram_tensor("wv", [128, D // 128 * GW], MMDT, kind="ExternalInput").ap()
    wo = nc.dram_tensor("wo", [128, GW // 128 * D], MMDT, kind="ExternalInput").ap()
    out = nc.dram_tensor("out", [S, D], F32, kind="ExternalOutput").ap()

    NT = S // 512          # 4 q/t ranges of 512
    NC = D // 128          # 8 contraction chunks for projections
    NJ = S // 128          # 16 k-chunks

    with tile.TileContext(nc) as tc, nc.allow_low_precision(reason="bf16 matmuls"):
        with (
            tc.tile_pool(name="const", bufs=1) as cpool,
            tc.tile_pool(name="xin", bufs=2) as xpool,
            tc.tile_pool(name="pt", bufs=8) as ppool,
            tc.tile_pool(name="small", bufs=6) as spool,
            tc.tile_pool(name="ost", bufs=6) as opool,
            tc.tile_pool(name="psum", bufs=1, space="PSUM") as psum,
        ):
            # ---- persistent tiles ----
            wq_sb = cpool.tile([128, NC, GW], MMDT)
            wk_sb = cpool.tile([128, NC, GW], MMDT)
            wv_sb = cpool.tile([128, NC, GW], MMDT)
            wo_sb = cpool.tile([128, 2, D], MMDT)

            QT = cpool.tile([128, 2, S], MMDT)   # [:, pair, t] feature-major
            KT = cpool.tile([128, 2, S], MMDT)
            Vt = cpool.tile([128, NJ, HPC * 65], MMDT)  # token-major + ones col
            ctxT = cpool.tile([128, 2, S], MMDT)

            # ones columns of V (col 64 of each 65-wide head slot)
            vt_ones = Vt[:, :, :].rearrange("p j (h u) -> p (j h) u", u=65)[:, :, 64:65]
            nc.vector.memset(vt_ones, 1.0)

            # triangular causal mask for the diagonal 128-block of scores^T:
            # keep (q - k >= 0) else -1e30   [partition = k, free = q]
            tri = cpool.tile([128, 128], F32, name="tri")
            nc.gpsimd.memset(tri[:], 0.0)
            nc.gpsimd.affine_select(
                out=tri[:],
                in_=tri[:],
                compare_op=mybir.AluOpType.is_ge,
                fill=NEG,
                base=0,
                pattern=[[1, 128]],
                channel_multiplier=-1,
            )

            # bf16 0/1 variant for post-exp masking on gpsimd
            tri01f = cpool.tile([128, 128], F32, name="tri01f")
            nc.gpsimd.memset(tri01f[:], 1.0)
            nc.gpsimd.affine_select(
                out=tri01f[:],
                in_=tri01f[:],
                compare_op=mybir.AluOpType.is_ge,
                fill=0.0,
                base=0,
                pattern=[[1, 128]],
                channel_multiplier=-1,
            )
            tri01 = cpool.tile([128, 128], MMDT, name="tri01")
            nc.scalar.copy(tri01[:], tri01f[:])

            # broadcast view of tri01 over the two stacked heads (0-stride dim)
            tri_ap = tri01[:]
            tri2 = bass.AP(
                tensor=tri_ap.tensor,
                offset=tri_ap.offset,
                ap=[list(tri_ap.ap[0]), [0, 2], list(tri_ap.ap[1])],
            )

            def load_xt(r):
                xt = xpool.tile([128, NC, 512], MMDT, tag="xt")
                nc.sync.dma_start(
                    xt[:],
                    xT[:, 512 * r : 512 * (r + 1)].rearrange("(c p) t -> p c t", p=128),
                )
                return xt

            def emit_wo(r):
                # output projection for q range r (runs one range behind so
                # its normalization dependency never stalls the PE queue)
                for qq in range(4):
                    qt = 4 * r + qq
                    for o in range(2):
                        po = psum.tile([128, 1024], F32, tag="mm", bufs=2)
                        for d in range(2):
                            nc.tensor.matmul(
                                po[:, 0:512],
                                ctxT[:, d, 128 * qt : 128 * (qt + 1)],
                                wo_sb[:, d, 512 * o : 512 * (o + 1)],
                                start=(d == 0), stop=(d == 1),
                            )
                        ot = opool.tile([128, 512], F32, tag="ot")
                        nc.scalar.copy(ot[:], po[:, 0:512])
                        nc.sync.dma_start(
                            out[128 * qt : 128 * (qt + 1), 512 * o : 512 * (o + 1)],
                            ot[:],
                        )

            xt_next = load_xt(0)
            for r in range(NT):
                # ---- projections for token range r ----
                xt = xt_next
                for w_sb, dst in ((wq_sb, QT), (wk_sb, KT)):
                    for o in range(2):
                        pm = psum.tile([128, 1024], F32, tag="mm", bufs=2)
                        for c in range(NC):
                            nc.tensor.matmul(
                                pm[:, 0:512],
                                w_sb[:, c, 128 * o : 128 * (o + 1)],
                                xt[:, c, :],
                                start=(c == 0),
                                stop=(c == NC - 1),
                            )
                        nc.vector.tensor_copy(
                            dst[:, o, 512 * r : 512 * (r + 1)], pm[:, 0:512]
                        )
                for tt in range(4):
                    j = 4 * r + tt
                    pv = psum.tile([128, 1024], F32, tag="mm", bufs=2)
                    for c in range(NC):
                        nc.tensor.matmul(
                            pv[:, 0:GW],
                            xt[:, c, 128 * tt : 128 * (tt + 1)],
                            wv_sb[:, c, :],
                            start=(c == 0),
                            stop=(c == NC - 1),
                        )
                    nc.vector.tensor_copy(
                        Vt[:, j, :].rearrange("p (h u) -> p h u", u=65)[:, :, 0:64],
                        pv[:, 0:GW].rearrange("p (h d) -> p h d", d=HD),
                    )

                # prefetch next range's x^T ahead of the output DMAs so the
                # in-order sync queue never holds it back
                if r + 1 < NT:
                    xt_next = load_xt(r + 1)

                # ---- attention for q range r (both head pairs) ----
                for p in range(2):
                    hA, hB = 2 * p, 2 * p + 1
                    nj = 4 * r + 4      # causal: k-chunks 0..nj-1
                    ca = psum.tile([65, 512], F32, tag="acc", bufs=4)
                    cb = psum.tile([65, 512], F32, tag="acc", bufs=4)
                    for j in range(nj):
                        # both heads' scores^T into one 2-bank PSUM tile
                        s2 = psum.tile([128, 1024], F32, tag="mm", bufs=2)
                        nc.tensor.matmul(
                            s2[:, 0:512],
                            KT[0:64, p, 128 * j : 128 * (j + 1)],
                            QT[0:64, p, 512 * r : 512 * (r + 1)],
                            start=True, stop=True,
                        )
                        nc.tensor.matmul(
                            s2[:, 512:1024],
                            KT[64:128, p, 128 * j : 128 * (j + 1)],
                            QT[64:128, p, 512 * r : 512 * (r + 1)],
                            start=True, stop=True,
                        )
                        pt2 = ppool.tile([128, 1024], MMDT, tag="pt")
                        s2v = s2[:, :].rearrange("p (s q) -> p s q", s=2)
                        pt2v = pt2[:, :].rearrange("p (s q) -> p s q", s=2)
                        v = j - 4 * r
                        if v >= 0:      # diagonal block inside this q-range
                            off = 128 * v
                            if off:     # left of diagonal: all-invalid -> 0
                                nc.gpsimd.memset(pt2v[:, :, 0:off], 0.0)
                            nc.scalar.activation(
                                pt2v[:, :, off:512], s2v[:, :, off:512],
                                EXPF, scale=SCALE,
                            )
                            # causal mask applied post-exp as 0/1 multiply on
                            # gpsimd (keeps DVE off the scores->exp chain)
                            nc.gpsimd.tensor_mul(
                                pt2v[:, :, off : off + 128],
                                pt2v[:, :, off : off + 128],
                                tri2,
                            )
                        else:
                            nc.scalar.activation(pt2[:], s2[:], EXPF, scale=SCALE)
                        nc.tensor.matmul(
                            ca[:], Vt[:, j, 65 * hA : 65 * hA + 65], pt2[:, 0:512],
                            start=(j == 0), stop=(j == nj - 1),
                        )
                        nc.tensor.matmul(
                            cb[:], Vt[:, j, 65 * hB : 65 * hB + 65], pt2[:, 512:1024],
                            start=(j == 0), stop=(j == nj - 1),
                        )
                    # normalize: ctxT = ctx_unnorm * (1/rowsum) broadcast (fp32)
                    ra = spool.tile([1, 512], F32, tag="rc")
                    rb = spool.tile([1, 512], F32, tag="rc")
                    nc.vector.reciprocal(ra[:], ca[64:65, :])
                    nc.vector.reciprocal(rb[:], cb[64:65, :])
                    bca = spool.tile([64, 512], F32, tag="bc")
                    bcb = spool.tile([64, 512], F32, tag="bc")
                    nc.gpsimd.partition_broadcast(bca[:], ra[:])
                    nc.gpsimd.partition_broadcast(bcb[:], rb[:])
                    qs = slice(512 * r, 512 * (r + 1))
                    nc.vector.tensor_mul(ctxT[0:64, p, qs], ca[0:64, :], bca[:])
                    nc.vector.tensor_mul(ctxT[64:128, p, qs], cb[0:64, :], bcb[:])

                # ---- output projection, one range behind ----
                if r > 0:
                    emit_wo(r - 1)
            emit_wo(NT - 1)

    nc.compile()
    return nc


def _get_nc():
    if "nc" not in _CACHE:
        _CACHE["nc"] = _build()
    return _CACHE["nc"]


def kernel(x, Wq, Wk, Wv, Wo, bo):
    global LAST_RESULTS
    x = np.asarray(x, dtype=np.float32)
    Wq = np.asarray(Wq, dtype=np.float32)
    Wk = np.asarray(Wk, dtype=np.float32)
    Wv = np.asarray(Wv, dtype=np.float32)
    Wo = np.asarray(Wo, dtype=np.float32)
    bo = np.asarray(bo, dtype=np.float32)

    nc = _get_nc()
    xTs = [np.ascontiguousarray(x[b].T).astype(NPDT) for b in range(B)]

    def warr(w, cs):
        # [D, GW] slice -> [128, NC*GW]: partition p holds chunk-major rows
        s = w[:, cs].reshape(D // 128, 128, GW).transpose(1, 0, 2)
        return np.ascontiguousarray(s.reshape(128, -1)).astype(NPDT)

    def woarr(cs):
        # [GW, D] slice -> [128, 2*D]
        s = Wo[cs, :].reshape(GW // 128, 128, D).transpose(1, 0, 2)
        return np.ascontiguousarray(s.reshape(128, -1)).astype(NPDT)

    in_maps = []
    for c in range(N_CORES):
        b, g = divmod(c, N_CORES // B)
        cs = slice(GW * g, GW * (g + 1))
        in_maps.append(
            {
                "xT": xTs[b],
                "wq": warr(Wq, cs),
                "wk": warr(Wk, cs),
                "wv": warr(Wv, cs),
                "wo": woarr(cs),
            }
        )

    _maybe_install_trace_hook()
    res = bass_utils.run_bass_kernel_spmd(nc, in_maps, core_ids=list(range(N_CORES)))
    LAST_RESULTS = res

    out = np.zeros((B, S, D), dtype=np.float32)
    for c in range(N_CORES):
        out[c // (N_CORES // B)] += res.results[c]["out"]
    out += bo[None, None, :]
    return out


# revision 25
# speedup vs baseline: 1.2378x; 1.2378x over previous
"""Trainium2 Bass kernel for multi-head causal attention.

Problem: B=2, S=2048, D=1024, H=16 heads (head_dim=64), fp32.
  q,k,v = x@Wq, x@Wk, x@Wv  (per-head split)
  scores = q@k^T, causal mask, softmax(scores/sqrt(64))
  out = (attn@v concat) @ Wo + bo

Sharding (8 cores): core c -> batch b=c//4, head group g=c%4 (4 heads).
Each core computes its 4 heads' attention plus the partial output
projection (row-parallel Wo); host sums 4 partials per batch and adds bo.

Layout strategy (zero on-device transposes):
 - x^T passed host-transposed (feature-major).
 - Q^T,K^T produced feature-major: (head_dim x tokens), two heads stacked
   per 128-partition tile; scores^T computed per 64-partition row group.
 - Both heads' scores^T tiles (k x q) land in one 2-bank PSUM tile so the
   causal mask memset + exp run as single wide instructions. The exp'd
   bf16 tile is directly the PV stationary operand. V is token-major with
   an appended ones-column so the PV matmul also emits the softmax
   denominators.
 - stage-and-release normalization: accumulators are copied to SBUF the
   moment a pair finishes (freeing PSUM banks in ~1us); the fp32 chain
   (partition-0 bounce -> fast reciprocal -> gpsimd partition broadcast
   -> DVE multiply) then runs off the critical path.
 - software-pipelined emission: projections for range r+1 are emitted
   before attention(r) and the output projection runs one range behind,
   so the PE never stalls on the softmax chain and the PE activity
   monitor stays at full clock.
Matmul operands are bf16 (1 cycle/row PE rate); every accumulation and
the softmax normalization stay fp32 in PSUM.
"""

import sys

sys.path.insert(0, "/opt/trn_rl_repo")

import ml_dtypes
import numpy as np

import concourse.bass as bass  # noqa: F401
import concourse.tile as tile
from concourse import bacc, bass_utils, mybir

F32 = mybir.dt.float32
MMDT = mybir.dt.bfloat16
NPDT = ml_dtypes.bfloat16
EXPF = mybir.ActivationFunctionType.Exp

B, S, D, H, HD = 2, 2048, 1024, 16, 64
N_CORES = 8
HPC = 4            # heads per core
GW = HPC * HD      # head-group width per core = 256
SCALE = 1.0 / np.sqrt(HD)
NEG = -1.0e30

_CACHE = {}
LAST_RESULTS = None


def _maybe_install_trace_hook():
    """If BASS_TRACE is set, bass_utils needs antenv.axon_hooks (absent in
    this image). Install it from trn_boot when possible; otherwise disable
    tracing so the run still works."""
    import os

    if not os.environ.get("BASS_TRACE"):
        return
    try:
        import antenv.axon_hooks  # noqa: F401
        return
    except ImportError:
        pass
    try:
        import types

        from trn_agent_boot.trn_boot import _ntff_profile_via_ctypes

        hook = _ntff_profile_via_ctypes("/opt/axon/libaxon_pjrt.so")
        mod = types.ModuleType("antenv.axon_hooks")
        mod.get_axon_ntff_profile_hook = lambda: hook
        mod.set_axon_ntff_profile_hook = lambda h: None
        import antenv

        sys.modules["antenv.axon_hooks"] = mod
        antenv.axon_hooks = mod
    except Exception:
        os.environ["BASS_NEVER_TRACE"] = "1"


def _build():
    nc = bacc.Bacc("TRN2", target_bir_lowering=False, debug=False)

    xT = nc.dram_tensor("xT", [D, S], MMDT, kind="ExternalInput").ap()
    wq = nc.dram_tensor("wq", [128, D // 128 * GW], MMDT, kind="ExternalInput").ap()
    wk = nc.dram_tensor("wk", [128, D // 128 * GW], MMDT, kind="ExternalInput").ap()
    wv = nc.dram_tensor("wv", [128, D // 128 * GW], MMDT, kind="ExternalInput").ap()
    wo = nc.dram_tensor("wo", [128, GW // 128 * D], MMDT, kind="ExternalInput").ap()
    out = nc.dram_tensor("out", [S, D], F32, kind="ExternalOutput").ap()

    NT = S // 512          # 4 q/t ranges of 512
    NC = D // 128          # 8 contraction chunks for projections
    NJ = S // 128          # 16 k-chunks

    with tile.TileContext(nc) as tc, nc.allow_low_precision(reason="bf16 matmuls"):
        with (
            tc.tile_pool(name="const", bufs=1) as cpool,
            tc.tile_pool(name="xin", bufs=2) as xpool,
            tc.tile_pool(name="pt", bufs=8) as ppool,
            tc.tile_pool(name="small", bufs=6) as spool,
            tc.tile_pool(name="ost", bufs=6) as opool,
            tc.tile_pool(name="psum", bufs=1, space="PSUM") as psum,
        ):
            # ---- persistent tiles ----
            wq_sb = cpool.tile([128, NC, GW], MMDT)
            wk_sb = cpool.tile([128, NC, GW], MMDT)
            wv_sb = cpool.tile([128, NC, GW], MMDT)
            wo_sb = cpool.tile([128, 2, D], MMDT)

            QT = cpool.tile([128, 2, S], MMDT)   # [:, pair, t] feature-major
            KT = cpool.tile([128, 2, S], MMDT)
            Vt = cpool.tile([128, NJ, HPC * 65], MMDT)  # token-major + ones col
            ctxT = cpool.tile([128, 2, S], MMDT)

            # ones columns of V (col 64 of each 65-wide head slot)
            vt_ones = Vt[:, :, :].rearrange("p j (h u) -> p (j h) u", u=65)[:, :, 64:65]
            nc.vector.memset(vt_ones, 1.0)

            # triangular causal mask for the diagonal 128-block of scores^T:
            # keep (q - k >= 0) else -1e30   [partition = k, free = q]
            tri = cpool.tile([128, 128], F32, name="tri")
            nc.gpsimd.memset(tri[:], 0.0)
            nc.gpsimd.affine_select(
                out=tri[:],
                in_=tri[:],
                compare_op=mybir.AluOpType.is_ge,
                fill=NEG,
                base=0,
                pattern=[[1, 128]],
                channel_multiplier=-1,
            )

            # broadcast view of tri over the two stacked heads (0-stride dim)
            tri_ap = tri[:]
            tri2 = bass.AP(
                tensor=tri_ap.tensor,
                offset=tri_ap.offset,
                ap=[list(tri_ap.ap[0]), [0, 2], list(tri_ap.ap[1])],
            )

            def load_xt(r):
                xt = xpool.tile([128, NC, 512], MMDT, tag="xt")
                nc.sync.dma_start(
                    xt[:],
                    xT[:, 512 * r : 512 * (r + 1)].rearrange("(c p) t -> p c t", p=128),
                )
                return xt

            def emit_wo(r):
                # output projection for q range r (runs one range behind so
                # its normalization dependency never stalls the PE queue)
                for qq in range(4):
                    qt = 4 * r + qq
                    for o in range(2):
                        po = psum.tile([128, 1024], F32, tag="mm", bufs=2)
                        for d in range(2):
                            nc.tensor.matmul(
                                po[:, 0:512],
                                ctxT[:, d, 128 * qt : 128 * (qt + 1)],
                                wo_sb[:, d, 512 * o : 512 * (o + 1)],
                                start=(d == 0), stop=(d == 1),
                            )
                        ot = opool.tile([128, 512], F32, tag="ot")
                        nc.scalar.copy(ot[:], po[:, 0:512])
                        nc.sync.dma_start(
                            out[128 * qt : 128 * (qt + 1), 512 * o : 512 * (o + 1)],
                            ot[:],
                        )

            xt_next = load_xt(0)
            for r in range(NT):
                # ---- projections for token range r ----
                xt = xt_next
                for w_sb, dst in ((wq_sb, QT), (wk_sb, KT)):
                    for o in range(2):
                        pm = psum.tile([128, 1024], F32, tag="mm", bufs=2)
                        for c in range(NC):
                            nc.tensor.matmul(
                                pm[:, 0:512],
                                w_sb[:, c, 128 * o : 128 * (o + 1)],
                                xt[:, c, :],
                                start=(c == 0),
                                stop=(c == NC - 1),
                            )
                        nc.vector.tensor_copy(
                            dst[:, o, 512 * r : 512 * (r + 1)], pm[:, 0:512]
                        )
                for tt in range(4):
                    j = 4 * r + tt
                    pv = psum.tile([128, 1024], F32, tag="mm", bufs=2)
                    for c in range(NC):
                        nc.tensor.matmul(
                            pv[:, 0:GW],
                            xt[:, c, 128 * tt : 128 * (tt + 1)],
                            wv_sb[:, c, :],
                            start=(c == 0),
                            stop=(c == NC - 1),
                        )
                    nc.vector.tensor_copy(
                        Vt[:, j, :].rearrange("p (h u) -> p h u", u=65)[:, :, 0:64],
                        pv[:, 0:GW].rearrange("p (h d) -> p h d", d=HD),
                    )

                # prefetch next range's x^T ahead of the output DMAs so the
                # in-order sync queue never holds it back
                if r + 1 < NT:
                    xt_next = load_xt(r + 1)

                # ---- attention for q range r (both head pairs) ----
                for p in range(2):
                    hA, hB = 2 * p, 2 * p + 1
                    nj = 4 * r + 4      # causal: k-chunks 0..nj-1
                    ca = psum.tile([65, 512], F32, tag="acc", bufs=4)
                    cb = psum.tile([65, 512], F32, tag="acc", bufs=4)
                    for j in range(nj):
                        # both heads' scores^T into one 2-bank PSUM tile
                        s2 = psum.tile([128, 1024], F32, tag="mm", bufs=2)
                        nc.tensor.matmul(
                            s2[:, 0:512],
                            KT[0:64, p, 128 * j : 128 * (j + 1)],
                            QT[0:64, p, 512 * r : 512 * (r + 1)],
                            start=True, stop=True,
                        )
                        nc.tensor.matmul(
                            s2[:, 512:1024],
                            KT[64:128, p, 128 * j : 128 * (j + 1)],
                            QT[64:128, p, 512 * r : 512 * (r + 1)],
                            start=True, stop=True,
                        )
                        pt2 = ppool.tile([128, 1024], MMDT, tag="pt")
                        s2v = s2[:, :].rearrange("p (s q) -> p s q", s=2)
                        pt2v = pt2[:, :].rearrange("p (s q) -> p s q", s=2)
                        v = j - 4 * r
                        if v >= 0:      # diagonal block inside this q-range
                            off = 128 * v
                            nc.vector.tensor_add(
                                s2v[:, :, off : off + 128],
                                s2v[:, :, off : off + 128],
                                tri2,
                            )
                            if off:     # left of diagonal: all-invalid -> 0
                                nc.gpsimd.memset(pt2v[:, :, 0:off], 0.0)
                            nc.scalar.activation(
                                pt2v[:, :, off:512], s2v[:, :, off:512],
                                EXPF, scale=SCALE,
                            )
                        else:
                            nc.scalar.activation(pt2[:], s2[:], EXPF, scale=SCALE)
                        nc.tensor.matmul(
                            ca[:], Vt[:, j, 65 * hA : 65 * hA + 65], pt2[:, 0:512],
                            start=(j == 0), stop=(j == nj - 1),
                        )
                        nc.tensor.matmul(
                            cb[:], Vt[:, j, 65 * hB : 65 * hB + 65], pt2[:, 512:1024],
                            start=(j == 0), stop=(j == nj - 1),
                        )
                    # normalize: ctxT = ctx_unnorm * (1/rowsum) broadcast (fp32)
                    ra = spool.tile([1, 512], F32, tag="rc")
                    rb = spool.tile([1, 512], F32, tag="rc")
                    nc.vector.reciprocal(ra[:], ca[64:65, :])
                    nc.vector.reciprocal(rb[:], cb[64:65, :])
                    bca = spool.tile([64, 512], F32, tag="bc")
                    bcb = spool.tile([64, 512], F32, tag="bc")
                    nc.gpsimd.partition_broadcast(bca[:], ra[:])
                    nc.gpsimd.partition_broadcast(bcb[:], rb[:])
                    qs = slice(512 * r, 512 * (r + 1))
                    nc.vector.tensor_mul(ctxT[0:64, p, qs], ca[0:64, :], bca[:])
                    nc.vector.tensor_mul(ctxT[64:128, p, qs], cb[0:64, :], bcb[:])

                # ---- output projection, one range behind ----
                if r > 0:
                    emit_wo(r - 1)
            emit_wo(NT - 1)

    nc.compile()
    return nc


def _get_nc():
    if "nc" not in _CACHE:
        _CACHE["nc"] = _build()
    return _CACHE["nc"]


def kernel(x, Wq, Wk, Wv, Wo, bo):
    global LAST_RESULTS
    x = np.asarray(x, dtype=np.float32)
    Wq = np.asarray(Wq, dtype=np.float32)
    Wk = np.asarray(Wk, dtype=np.float32)
    Wv = np.asarray(Wv, dtype=np.float32)
    Wo = np.asarray(Wo, dtype=np.float32)
    bo = np.asarray(bo, dtype=np.float32)

    nc = _get_nc()
    xTs = [np.ascontiguousarray(x[b].T).astype(NPDT) for b in range(B)]

    def warr(w, cs):
        # [D, GW] slice -> [128, NC*GW]: partition p holds chunk-major rows
        s = w[:, cs].reshape(D // 128, 128, GW).transpose(1, 0, 2)
        return np.ascontiguousarray(s.reshape(128, -1)).astype(NPDT)

    def woarr(cs):
        # [GW, D] slice -> [128, 2*D]
        s = Wo[cs, :].reshape(GW // 128, 128, D).transpose(1, 0, 2)
        return np.ascontiguousarray(s.reshape(128, -1)).astype(NPDT)

    in_maps = []
    for c in range(N_CORES):
        b, g = divmod(c, N_CORES // B)
        cs = slice(GW * g, GW * (g + 1))
        in_maps.append(
            {
                "xT": xTs[b],
                "wq": warr(Wq, cs),
                "wk": warr(Wk, cs),
                "wv": warr(Wv, cs),
                "wo": woarr(cs),
            }
        )

    _maybe_install_trace_hook()
    res = bass_utils.run_bass_kernel_spmd(nc, in_maps, core_ids=list(range(N_CORES)))
    LAST_RESULTS = res

    out = np.zeros((B, S, D), dtype=np.float32)
    for c in range(N_CORES):
        out[c // (N_CORES // B)] += res.results[c]["out"]
    out += bo[None, None, :]
    return out
